# revision 2
# baseline (speedup 1.0000x reference)
import os, sys
import numpy as np

sys.path.insert(0, '/opt/trn_rl_repo')
from contextlib import ExitStack
import concourse.bass as bass
import concourse.tile as tile
from concourse import bacc, mybir
from concourse import bass_utils

F32 = mybir.dt.float32
BF = mybir.dt.float16
AF = mybir.ActivationFunctionType
ALU = mybir.AluOpType
AX = mybir.AxisListType

NPBF = mybir.dt.np(BF)

S, B, E, H = 2048, 64, 256, 256
KN = 256
OUT = 10
NC = 8
V = 65                  # vocab 64 + identity pad token
W = 24                  # warmup halo steps
CH = 128                # seq chunk per chain
N = CH + W              # 176 steps per chain
BN = 4                  # idx block steps
NBLK = N // BN          # 22
SL = 256                # seq per core
H3 = 3 * H

TAPS = [(ki, k, j) for ki, k in enumerate((3, 4, 5)) for j in range(k)]
# paired conv tap slots: (ki, slot) -> flat slot index; slot covers taps 2s,2s+1
PSLOTS = [(ki, sl) for ki, k in enumerate((3, 4, 5)) for sl in range((k + 1) // 2)]
PSLOT = {p: i for i, p in enumerate(PSLOTS)}

_cache = {}


def _build():
    nc = bacc.Bacc("TRN2", target_bir_lowering=False, debug=False)
    idxf = nc.dram_tensor("idxf", (V, N * 128), BF, kind="ExternalInput")
    idxb = nc.dram_tensor("idxb", (V, N * 128), BF, kind="ExternalInput")
    idxc = nc.dram_tensor("idxc", (V, SL * 64 + 8), BF, kind="ExternalInput")
    gtabf = nc.dram_tensor("gtabf", (V, H3), BF, kind="ExternalInput")
    gtabb = nc.dram_tensor("gtabb", (V, H3), BF, kind="ExternalInput")
    whhf = nc.dram_tensor("whhf", (H, H3), BF, kind="ExternalInput")
    whhb = nc.dram_tensor("whhb", (H, H3), BF, kind="ExternalInput")
    bnrow = nc.dram_tensor("bnrow", (1, 512), BF, kind="ExternalInput")
    h0f = nc.dram_tensor("h0f", (128, 256), BF, kind="ExternalInput")
    h0b = nc.dram_tensor("h0b", (128, 256), BF, kind="ExternalInput")
    ut = nc.dram_tensor("ut", (128, 14 * 128), BF, kind="ExternalInput")
    convb = nc.dram_tensor("convb", (128, 6), F32, kind="ExternalInput")
    lwt = nc.dram_tensor("lwt", (128, 6 * 512), BF, kind="ExternalInput")
    lbias = nc.dram_tensor("lbias", (1, 512), BF, kind="ExternalInput")
    wwt = nc.dram_tensor("wwt", (128, 4 * 512), BF, kind="ExternalInput")
    bwordr = nc.dram_tensor("bwordr", (1, 512), BF, kind="ExternalInput")
    fct = nc.dram_tensor("fct", (128, 4 * 640), BF, kind="ExternalInput")
    attn = nc.dram_tensor("attn", (128, 128), F32, kind="ExternalOutput")
    gout = nc.dram_tensor("gout", (128, 1280), F32, kind="ExternalOutput")

    with tile.TileContext(nc) as tc, ExitStack() as ctx:
        consts = ctx.enter_context(tc.tile_pool(name="consts", bufs=1))

        # persistent state
        ow_t = consts.tile([128, 4 * 16384], BF, tag="ow", name="ow")
        wprojS = consts.tile([128, 1024], F32, tag="wprojS", name="wprojS")
        attn_sb = consts.tile([128, 128], F32, tag="attnsb", name="attnsb")
        g_sb = consts.tile([128, 1280], F32, tag="gsb", name="gsb")
        iota_i = consts.tile([V, 1], mybir.dt.int32, tag="iotai", name="iotai")
        nc.gpsimd.iota(iota_i[:], [[0, 1]], base=0, channel_multiplier=1)
        iota_bf = consts.tile([V, 1], F32, tag="iotab", name="iotab")
        nc.vector.tensor_copy(iota_bf[:], iota_i[:])
        ones_bf = consts.tile([1, 128], BF, tag="ones", name="ones")
        nc.vector.memset(ones_bf[:], 1.0)

        f_t = consts.tile([128, 6 * SL], BF, tag="ft", name="ft")
        # 4-d view of ow: col = c*16384 + b*256 + s
        # step columns everywhere are (b, chain) b-major, so slicing
        # [:, c, :, s::128] yields free dims (b, chain) matching psum cols.
        ow_cbs = ow_t[:].rearrange("p (c b s) -> p c b s", c=4, b=64, s=256)

        phases = os.environ.get("KPHASES", "cnn,gru,s2").split(",")
        if "gru" not in phases and "s2" in phases:
            nc.vector.memset(ow_t[:], 0.0)
        if "cnn" not in phases:
            nc.vector.memset(wprojS[:], 0.0)
        if "s2" not in phases:
            nc.vector.memset(attn_sb[:], 0.0)
            nc.vector.memset(g_sb[:], 0.0)

        # -------- Merged phase: GRU rounds with CNN units interleaved ------
        with tc.tile_pool(name="gruc", bufs=1) as gruc, \
             tc.tile_pool(name="gidx", bufs=2) as gidx, \
             tc.tile_pool(name="goh", bufs=2) as goh, \
             tc.tile_pool(name="gwork", bufs=2) as gwork, \
             tc.tile_pool(name="cnnio", bufs=2) as cnnio, \
             tc.tile_pool(name="cnnw", bufs=2) as cnnw, \
             tc.tile_pool(name="psrz", bufs=1, space="PSUM") as psrz, \
             tc.tile_pool(name="psng", bufs=1, space="PSUM") as psng, \
             tc.tile_pool(name="cnps", bufs=2, space="PSUM") as cnps:
            # ---- consts ----
            ut_t = gruc.tile([128, 14 * 128], BF, tag="ut", name="ut")
            nc.sync.dma_start(ut_t[:], ut.ap())
            convb_t = gruc.tile([128, 6], F32, tag="cvb", name="cvb")
            nc.sync.dma_start(convb_t[:], convb.ap())

            gt = {}
            wh = {}
            for d, (gsrc, wsrc) in enumerate(((gtabf, whhf), (gtabb, whhb))):
                gt[d] = gruc.tile([V, H3], BF, tag=f"gt{d}", name=f"gt{d}")
                nc.sync.dma_start(gt[d][:], gsrc.ap())
                wh[d] = [gruc.tile([128, H3], BF, tag=f"wh{d}{kk}", name=f"wh{d}{kk}")
                         for kk in range(2)]
                for kk in range(2):
                    nc.sync.dma_start(wh[d][kk][:], wsrc.ap()[kk * 128:(kk + 1) * 128, :])
            bnrow_t = gruc.tile([1, 512], BF, tag="bnr", name="bnr")
            nc.sync.dma_start(bnrow_t[:], bnrow.ap())
            h0t = {}
            for d, hsrc in enumerate((h0f, h0b)):
                h0t[d] = gruc.tile([128, 256], BF, tag=f"h0{d}", name=f"h0{d}")
                nc.sync.dma_start(h0t[d][:], hsrc.ap())
            # rotating h state: f32 master + fp16 copy for matmul rhs (warmup)
            scr32 = {d: [gruc.tile([128, 256], F32, tag=f"sf{d}{q}", name=f"sf{d}{q}")
                         for q in range(2)] for d in range(2)}
            scr16 = {d: [gruc.tile([128, 256], BF, tag=f"sh{d}{q}", name=f"sh{d}{q}")
                         for q in range(2)] for d in range(2)}

            oh_blk = {0: None, 1: None}

            def load_blocks(ib):
                for d, src in ((0, idxf), (1, idxb)):
                    ix = gidx.tile([V, BN * 128], BF, tag=f"gix{d}")
                    nc.sync.dma_start(ix[:], src.ap()[:, ib * BN * 128:(ib + 1) * BN * 128])
                    oh = goh.tile([V, BN * 128], BF, tag=f"goh{d}")
                    nc.gpsimd.tensor_scalar(oh[:], ix[:], iota_bf[:, 0:1], None,
                                            ALU.is_equal)
                    oh_blk[d] = oh

            def hprev_ap(d, i):
                # returns (f32 full 3d ap for elementwise, fp16 2d aps for mms)
                if i == 0:
                    t = h0t[d][:]
                    full = t.rearrange("p (k b c) -> p k b c", k=2, b=64, c=2)
                    return full, [t[:, 0:128], t[:, 128:256]]
                full = scr32[d][(i - 1) % 2][:].rearrange(
                    "p (k b c) -> p k b c", k=2, b=64, c=2)
                t = scr16[d][(i - 1) % 2][:]
                return full, [t[:, 0:128], t[:, 128:256]]

            def v3(ap):
                return ap.rearrange("p (k b c) -> p k b c", k=2, b=64, c=2)

            def step(d, i):
                oh = oh_blk[d][:, (i % BN) * 128:(i % BN + 1) * 128]
                hfull, hkk = hprev_ap(d, i)
                prz = psrz.tile([128, 512], F32, tag=f"prz{d}")
                rz = gwork.tile([128, 512], F32, tag=f"rz{d}")
                # r,z region groups: [gx, whh0, whh1] each; r first so sigma(r)
                # can issue as early as possible
                for cc in range(4):
                    nc.tensor.matmul(prz[:, cc * 128:(cc + 1) * 128],
                                     gt[d][:, cc * 128:(cc + 1) * 128],
                                     oh, start=True, stop=False)
                    for kk in range(2):
                        nc.tensor.matmul(prz[:, cc * 128:(cc + 1) * 128],
                                         wh[d][kk][:, cc * 128:(cc + 1) * 128],
                                         hkk[kk], start=False, stop=(kk == 1))
                    if cc == 1:
                        nc.scalar.activation(rz[:, 0:256], prz[:, 0:256],
                                             AF.Sigmoid)
                nc.scalar.activation(rz[:, 256:512], prz[:, 256:512], AF.Sigmoid)
                png = psng.tile([128, 512], F32, tag=f"png{d}")
                # oh-only gxn groups (regions 2,3 of png)
                for cc2 in range(2):
                    nc.tensor.matmul(png[:, (2 + cc2) * 128:(3 + cc2) * 128],
                                     gt[d][:, (4 + cc2) * 128:(5 + cc2) * 128],
                                     oh, start=True, stop=True)
                # n region groups: [bias-row, whh0, whh1]
                for cc2 in range(2):
                    nc.tensor.matmul(png[:, cc2 * 128:(cc2 + 1) * 128],
                                     bnrow_t[:, (d * 2 + cc2) * 128:(d * 2 + cc2) * 128 + 128],
                                     ones_bf[:], start=True, stop=False)
                    for kk in range(2):
                        nc.tensor.matmul(png[:, cc2 * 128:(cc2 + 1) * 128],
                                         wh[d][kk][:, (4 + cc2) * 128:(5 + cc2) * 128],
                                         hkk[kk], start=False, stop=(kk == 1))
                # zc = 1 - z on gpsimd: (z - 1) * -1
                zc = gwork.tile([128, 256], BF, tag=f"zc{d}")
                nc.gpsimd.tensor_scalar(zc[:], rz[:, 256:512], 1.0, -1.0,
                                        ALU.subtract, ALU.mult)
                # u = z * h_prev  (gpsimd, off the critical path)
                u = gwork.tile([128, 256], F32, tag=f"u{d}")
                nc.gpsimd.tensor_mul(v3(u[:]), v3(rz[:, 256:512]), hfull)
                rghn = gwork.tile([128, 256], F32, tag=f"rg{d}")
                nc.vector.tensor_mul(rghn[:], rz[:, 0:256], png[:, 0:256])
                prn = gwork.tile([128, 256], F32, tag=f"pn{d}")
                nc.vector.tensor_add(prn[:], rghn[:], png[:, 256:512])
                nt = gwork.tile([128, 256], F32, tag=f"nt{d}")
                nc.scalar.activation(nt[:], prn[:], AF.Tanh)
                v = gwork.tile([128, 256], F32, tag=f"v{d}")
                nc.gpsimd.tensor_mul(v[:], zc[:], nt[:])
                # parallel adds: fp16 for next-step matmuls, f32 master state
                nc.gpsimd.tensor_add(scr16[d][i % 2][:], u[:], v[:])
                hc = scr32[d][i % 2]
                nc.gpsimd.tensor_add(hc[:], u[:], v[:])
                if i >= W:
                    if d == 0:
                        sc = i - W
                    else:
                        sc = 127 - (i - W)
                    dest = ow_cbs[:, 2 * d:2 * d + 2, :, sc:sc + 129:128]
                    nc.gpsimd.tensor_copy(dest, v3(hc[:]))

            # ---- CNN unit machinery ----
            NUNITS = (SL // 8) * 6
            cnn_units = [(blk, ki, k, m) for blk in range(SL // 8)
                         for ki, k in ((0, 3), (1, 4), (2, 5)) for m in range(2)]
            cnn_state = {"ohc": None}

            def cnn_unit(idx):
                blk, ki, k, m = cnn_units[idx]
                if idx % 6 == 0:
                    ixc = cnnio.tile([64, 520], BF, tag="ixc", name="ixc")
                    nc.sync.dma_start(ixc[:], idxc.ap()[0:64, blk * 512: blk * 512 + 520])
                    ohc = cnnio.tile([128, 520], BF, tag="ohc", name="ohc")
                    nc.gpsimd.tensor_scalar(ohc[0:64, :], ixc[:], iota_bf[0:64, 0:1],
                                            None, ALU.is_equal)
                    # rows 64:128 = one-hot shifted left by one column (tap j+1)
                    nc.gpsimd.tensor_copy(ohc[64:128, 0:519], ohc[0:64, 1:520])
                    cnn_state["ohc"] = ohc
                ohc = cnn_state["ohc"]
                ci = ki * 2 + m
                yp = cnps.tile([128, 512], F32, tag="yp")
                nslot = (k + 1) // 2
                for sl2 in range(nslot):
                    t = PSLOT[(ki, sl2)]
                    jb = sl2 * 2
                    nc.tensor.matmul(
                        yp[:], ut_t[:, (t * 2 + m) * 128:(t * 2 + m) * 128 + 128],
                        ohc[:, jb: jb + 512],
                        start=(sl2 == 0), stop=(sl2 == nslot - 1))
                yr = cnnw.tile([128, 512], BF, tag="yr")
                nc.scalar.activation(yr[:], yp[:], AF.Relu,
                                     bias=convb_t[:, ci:ci + 1])
                y3 = yr[:].rearrange("p (s l) -> p s l", l=64)
                L = 64 - k + 1
                nc.gpsimd.memset(y3[:, :, L:64], 0.0)
                nc.vector.tensor_reduce(
                    f_t[:, ci * SL + blk * 8: ci * SL + (blk + 1) * 8],
                    y3, AX.X, ALU.max)

            # ---- round loop ----
            do_gru = "gru" in phases
            do_cnn = "cnn" in phases
            emitted = 0
            for i in range(N if do_gru else 0):
                if i % BN == 0:
                    load_blocks(i // BN)
                step(0, i)
                step(1, i)
                if do_cnn:
                    target = ((i + 1) * NUNITS) // N
                    while emitted < target:
                        cnn_unit(emitted)
                        emitted += 1
            if do_cnn:
                while emitted < NUNITS:
                    cnn_unit(emitted)
                    emitted += 1

        # ---------------- Phase C: attention + fc ----------------
        with tc.tile_pool(name="s2c", bufs=1) as s2c, \
             tc.tile_pool(name="s2w", bufs=3) as s2w, \
             tc.tile_pool(name="sqps", bufs=3, space="PSUM") as sqps, \
             tc.tile_pool(name="gps", bufs=3, space="PSUM") as gps:
            lwt_t = s2c.tile([128, 6 * 512], BF, tag="lwt", name="lwt")
            nc.sync.dma_start(lwt_t[:], lwt.ap())
            lb_row = s2c.tile([1, 512], BF, tag="lbr", name="lbr")
            nc.sync.dma_start(lb_row[:], lbias.ap())
            wwt_t = s2c.tile([128, 4 * 512], BF, tag="wwt", name="wwt")
            nc.sync.dma_start(wwt_t[:], wwt.ap())
            bw_row = s2c.tile([1, 512], BF, tag="bwr", name="bwr")
            nc.sync.dma_start(bw_row[:], bwordr.ap())
            fct_t = s2c.tile([128, 4 * 640], BF, tag="fct", name="fct")
            nc.sync.dma_start(fct_t[:], fct.ap())
            ow_cn = ow_t[:].rearrange("p (c n) -> p c n", c=4)

            if "cnn" in phases:
                # wproj (s-part, proj-col layout), bias via ones-row matmul
                for sh in range(2):
                    wp = sqps.tile([128, 512], F32, tag="sq")
                    for cci in range(6):
                        nc.tensor.matmul(wp[:], f_t[:, cci * SL + sh * 128: cci * SL + sh * 128 + 128],
                                         lwt_t[:, cci * 512:(cci + 1) * 512],
                                         start=(cci == 0), stop=False)
                    nc.tensor.matmul(wp[:], ones_bf[:], lb_row[:],
                                     start=False, stop=True)
                    nc.vector.tensor_copy(wprojS[:, sh * 512:(sh + 1) * 512], wp[:])

            for t in range(128 if "s2" in phases else 0):
                b = t // 2
                sh = t % 2
                col0 = b * 256 + sh * 128
                sq = sqps.tile([128, 512], F32, tag="sq")
                for kk in range(4):
                    nc.tensor.matmul(sq[:], ow_cn[:, kk, col0:col0 + 128],
                                     wwt_t[:, kk * 512:(kk + 1) * 512],
                                     start=(kk == 0), stop=False)
                nc.tensor.matmul(sq[:], ones_bf[:], bw_row[:],
                                 start=False, stop=True)
                sqt = s2w.tile([128, 512], F32, tag="sqt")
                nc.scalar.activation(sqt[:], sq[:], AF.Tanh)
                nc.vector.scalar_tensor_tensor(
                    sqt[:], sqt[:], 1.0, wprojS[:, sh * 512:(sh + 1) * 512],
                    ALU.mult, ALU.mult,
                    accum_out=attn_sb[:, t:t + 1])
                gp = gps.tile([128, OUT], F32, tag="gp")
                for kk in range(4):
                    nc.tensor.matmul(gp[:], ow_cn[:, kk, col0:col0 + 128],
                                     fct_t[:, kk * 640 + b * OUT: kk * 640 + (b + 1) * OUT],
                                     start=(kk == 0), stop=(kk == 3))
                nc.vector.tensor_copy(g_sb[:, t * OUT:(t + 1) * OUT], gp[:])

            nc.sync.dma_start(attn.ap(), attn_sb[:])
            nc.sync.dma_start(gout.ap(), g_sb[:])

    nc.compile()
    return nc


def prep_in_maps(embed, state_word, lookup,
                 W_ih_f, W_hh_f, b_ih_f, b_hh_f,
                 W_ih_b, W_hh_b, b_ih_b, b_hh_b,
                 W_word, b_word,
                 conv_w3, conv_b3, conv_w4, conv_b4, conv_w5, conv_b5,
                 cnn_lin_w, cnn_lin_b, fc_w, fc_b):
    f32 = np.float32
    embed = np.asarray(embed)
    state_word = np.asarray(state_word, f32)
    lookup = np.asarray(lookup, f32)

    # ---- shared host prep ----
    def bf(x):
        return np.ascontiguousarray(np.asarray(x, f32)).astype(NPBF)

    gtabs = {}
    whhs = {}
    for d, (W_ih, W_hh, b_ih, b_hh) in enumerate((
            (W_ih_f, W_hh_f, b_ih_f, b_hh_f),
            (W_ih_b, W_hh_b, b_ih_b, b_hh_b))):
        W_ih = np.asarray(W_ih, f32); W_hh = np.asarray(W_hh, f32)
        b_ih = np.asarray(b_ih, f32); b_hh = np.asarray(b_hh, f32)
        G = W_ih @ lookup.T + b_ih[:, None]
        G[0:2 * H] += b_hh[0:2 * H, None]
        g65 = np.zeros((V, H3), f32)
        g65[0:64] = G.T
        g65[64, H:2 * H] = 30.0      # pad token: z=1 -> identity step
        gtabs[d] = bf(g65)
        whhs[d] = bf(W_hh.T)         # (256, 768)
    b_hh_f = np.asarray(b_hh_f, f32); b_hh_b = np.asarray(b_hh_b, f32)
    bnrow = np.concatenate([b_hh_f[2 * H:], b_hh_b[2 * H:]])[None, :]  # (1,512)
    bnrow = bf(bnrow)

    ut = np.zeros((128, 14 * 128), f32)
    for t, (ki, sl) in enumerate(PSLOTS):
        w = np.asarray((conv_w3, conv_w4, conv_w5)[ki], f32)
        k = w.shape[2]
        for half in range(2):
            j = sl * 2 + half
            if j >= k:
                continue
            U = lookup @ w[:, :, j].T    # (64, 256)
            for m in range(2):
                ut[half * 64:(half + 1) * 64,
                   (t * 2 + m) * 128:(t * 2 + m) * 128 + 128] = U[:, m * 128:(m + 1) * 128]
    ut = bf(ut)
    convb = np.zeros((128, 6), f32)
    for ki, cb in enumerate((conv_b3, conv_b4, conv_b5)):
        cb = np.asarray(cb, f32)
        convb[:, ki * 2] = cb[0:128]
        convb[:, ki * 2 + 1] = cb[128:256]
    lwt = np.asarray(cnn_lin_w, f32).T                       # (768, 512)
    lwt = bf(lwt.reshape(6, 128, 512).transpose(1, 0, 2).reshape(128, 6 * 512))
    lbias = bf(np.asarray(cnn_lin_b, f32)[None, :])
    wwt = np.asarray(W_word, f32)                            # (512, 512)
    wwt = bf(wwt.reshape(4, 128, 512).transpose(1, 0, 2).reshape(128, 4 * 512))
    bwordr = bf(np.asarray(b_word, f32)[:, 0][None, :])
    fcw = np.asarray(fc_w, f32).reshape(OUT, 64, 512)        # (o, b, h)
    fctm = np.transpose(fcw, (2, 1, 0)).reshape(4, 128, 64 * OUT)  # (kk,128,b*o)
    fct = bf(np.ascontiguousarray(fctm.transpose(1, 0, 2).reshape(128, 4 * 640)))

    in_maps = []
    for c in range(NC):
        s0 = c * SL
        # step columns are (b, chain) b-major within each 128-col step
        si = np.arange(N) - W
        sf = np.stack([s0 + si, s0 + 128 + si], axis=1)       # (N, ch)
        sb = np.stack([s0 + 127 + W - np.arange(N),
                       s0 + 255 + W - np.arange(N)], axis=1)  # (N, ch)
        def toks(sm):
            t = np.full((N, 2, B), 64, np.int64)              # (i, ch, b)
            ok = (sm >= 0) & (sm < S)
            t[ok] = embed[sm[ok]]
            return np.ascontiguousarray(t.transpose(0, 2, 1)).reshape(N * 128)
        idxf_ = np.broadcast_to(toks(sf).astype(f32)[None, :], (V, N * 128))
        idxb_ = np.broadcast_to(toks(sb).astype(f32)[None, :], (V, N * 128))
        idxc_ = np.full(SL * 64 + 8, 64.0, f32)
        idxc_[:SL * 64] = embed[s0:s0 + SL].astype(f32).ravel()
        idxc_ = np.broadcast_to(idxc_[None, :], (V, SL * 64 + 8))

        h0s = {}
        for d in range(2):
            h0 = np.zeros((128, 2, 64, 2), f32)   # (p, kk, b, ch)
            if d == 0 and c == 0:
                hT = state_word[0].T               # (256, 64)
                h0[:, 0, :, 0] = hT[0:128]
                h0[:, 1, :, 0] = hT[128:256]
            if d == 1 and c == NC - 1:
                hT = state_word[1].T
                h0[:, 0, :, 1] = hT[0:128]
                h0[:, 1, :, 1] = hT[128:256]
            h0s[d] = bf(h0.reshape(128, 256))

        in_maps.append({
            "idxf": idxf_.astype(NPBF), "idxb": idxb_.astype(NPBF),
            "idxc": idxc_.astype(NPBF),
            "gtabf": gtabs[0], "gtabb": gtabs[1],
            "whhf": whhs[0], "whhb": whhs[1],
            "bnrow": bnrow, "h0f": h0s[0], "h0b": h0s[1],
            "ut": ut, "convb": convb, "lwt": lwt, "lbias": lbias,
            "wwt": wwt, "bwordr": bwordr, "fct": fct,
        })
    return in_maps


def kernel(embed, state_word, lookup,
           W_ih_f, W_hh_f, b_ih_f, b_hh_f,
           W_ih_b, W_hh_b, b_ih_b, b_hh_b,
           W_word, b_word,
           conv_w3, conv_b3, conv_w4, conv_b4, conv_w5, conv_b5,
           cnn_lin_w, cnn_lin_b, fc_w, fc_b):
    f32 = np.float32
    trace = os.environ.get("KTRACE") == "1"
    if "m" not in _cache:
        _cache["m"] = _build()
    in_maps = prep_in_maps(
        embed, state_word, lookup,
        W_ih_f, W_hh_f, b_ih_f, b_hh_f,
        W_ih_b, W_hh_b, b_ih_b, b_hh_b,
        W_word, b_word,
        conv_w3, conv_b3, conv_w4, conv_b4, conv_w5, conv_b5,
        cnn_lin_w, cnn_lin_b, fc_w, fc_b)

    import time as _t
    _t0 = _t.time()
    if os.environ.get("KSIM") == "1":
        from types import SimpleNamespace
        from concourse.bass_interp import CoreSim
        results = []
        cores = [int(x) for x in os.environ.get("KSIM_CORES", "0,3").split(",")]
        est = None
        for c in range(NC):
            if c not in cores:
                results.append({"attn": np.zeros((128, 128), f32),
                                "gout": np.zeros((128, 1280), f32)})
                continue
            sim = CoreSim(_cache["m"])
            for k2, v2 in in_maps[c].items():
                sim.tensor(k2)[:] = v2
            sim.simulate(check_with_hw=False)
            est = getattr(sim, "time", None)
            results.append({"attn": np.array(sim.tensor("attn"), f32),
                            "gout": np.array(sim.tensor("gout"), f32)})
        print("sim est time ns:", est)
        r = SimpleNamespace(results=results, exec_time_ns=est)
    else:
        try:
            r = bass_utils.run_bass_kernel_spmd(_cache["m"], in_maps,
                                                core_ids=list(range(NC)), trace=trace)
        except ModuleNotFoundError:
            # axon NTFF profiling hook unavailable: run without trace
            r = bass_utils.run_bass_kernel_spmd(_cache["m"], in_maps,
                                                core_ids=list(range(NC)), trace=False)
    kernel.wall = [_t.time() - _t0]
    kernel.exec_ns = [r.exec_time_ns]

    # ---- host: softmax + combine ----
    attn = np.empty((S, B), f32)
    g = np.empty((S, B, OUT), f32)
    for c in range(NC):
        a = r.results[c]["attn"]                   # (128, 128): [p, b*2+sh]
        a = a.reshape(128, 64, 2)                  # (p, b, sh)
        attn[c * SL:(c + 1) * SL] = a.transpose(2, 0, 1).reshape(SL, B)
        gt = r.results[c]["gout"].reshape(128, 64, 2, OUT)
        g[c * SL:(c + 1) * SL] = gt.transpose(2, 0, 1, 3).reshape(SL, B, OUT)
    a = attn - attn.max(axis=0, keepdims=True)
    ea = np.exp(a)
    an = ea / ea.sum(axis=0, keepdims=True)
    logits = np.einsum('sb,sbo->so', an, g) + np.asarray(fc_b, f32)
    z = logits - logits.max(axis=-1, keepdims=True)
    ez = np.exp(z)
    return (ez / ez.sum(axis=-1, keepdims=True)).astype(f32)
